# revision 16
# baseline (speedup 1.0000x reference)
"""LongFormer dilated-window attention block on 8 trn2 NeuronCores.

Sharding: 8 cores = 4 dilation residues x 2 sequence halves. Query q
attends keys q + 4*j - 512 (j=0..256), i.e. only keys with the same
residue mod DIL=4. De-interleaving by residue turns the dilated window
into a contiguous +-128 sliding window in "residue space". Each core
gets a zero-padded [512, 512] slice of x: its 256 owned rows plus a
128-row halo on each side (all in residue space), so no communication
is needed.

v2 design (vs the fp32 v1 baseline):
- Host pre-casts x and all weights to fp16 and pre-packs them into the
  exact SBUF layouts (x pre-transposed to [e, seq]; weights pre-tiled
  per stationary 128x128 tile). Halves HBM traffic, turns every DMA
  into contiguous >=1KB partition lines, and eliminates all on-chip PE
  transposes and the identity constant.
- All matmuls run fp16 (bf16 for softmax probabilities, which can
  reach exp(44) and overflow fp16). fp16/bf16 weights enable fast
  weight load; fp32 weights do not.
- Scores/p@v are trimmed to the 3 valid 128-key diagonal blocks per
  128-query block (the +-128 window spans 384 keys, not 512).
- Both heads of a pair pack into one PSUM bank: scores as one
  2-row-group matmul group [keys, 2, q]; p@v as even rows 0:64 /
  odd rows 64:128 (col-grp tile_position).
- Softmax denominators via ones-vector matmuls into a [1, 512] bank;
  normalization = DVE reciprocal -> gpsimd partition_broadcast -> DVE
  muls (no PE broadcast/shift matmuls). 1/sqrt(d) is folded into W1
  host-side (FFN1 is linear in x1).
- FFN runs fully transposed ([f, q] layouts) so FFN1's output feeds
  FFN2 directly with no transposes; output is written transposed and
  the host transposes it back. Residual comes from the x^T tile.
- A few warm-up matmuls on a zeroed tile run during the input DMAs to
  lift the PE out of its cold 1.2GHz HAM state before real work.
"""

import sys

if "/opt/trn_rl_repo" not in sys.path:
    sys.path.insert(0, "/opt/trn_rl_repo")

import numpy as np

N_CORES = 8
S, E, H, FEAT = 2048, 512, 8, 64
DIL = 4
SC = 256      # owned queries per core (residue space)
EXT = 512     # ext rows per core (owned + 128 halo each side)

_CACHE = {}


def _build_nc():
    import concourse.bacc as bacc
    import concourse.tile as tile
    import concourse.mybir as mybir
    import concourse.bass as bass

    dt = mybir.dt
    f32 = dt.float32
    f16 = dt.float16
    bf16 = dt.bfloat16
    Alu = mybir.AluOpType
    Act = mybir.ActivationFunctionType

    nc = bacc.Bacc("TRN2", target_bir_lowering=False, debug=False,
                   num_devices=N_CORES)

    # ---- DRAM I/O (host pre-packed, fp16) ----
    xeT_d = nc.dram_tensor("xeT", [128, 4, EXT], f16, kind="ExternalInput").ap()
    wq_d = nc.dram_tensor("wq", [4, 128, 4, 128], f16, kind="ExternalInput").ap()
    wk_d = nc.dram_tensor("wk", [4, 128, 4, 128], f16, kind="ExternalInput").ap()
    wv_d = nc.dram_tensor("wv", [128, 4, E], f16, kind="ExternalInput").ap()
    w1_d = nc.dram_tensor("w1", [4, 128, 4, 128], f16, kind="ExternalInput").ap()
    w2_d = nc.dram_tensor("w2", [128, 4, 4, 128], f16, kind="ExternalInput").ap()
    b1_d = nc.dram_tensor("b1", [128, 4], f32, kind="ExternalInput").ap()
    b2_d = nc.dram_tensor("b2", [128, 4], f32, kind="ExternalInput").ap()
    out_d = nc.dram_tensor("out", [4, 128, SC], f32, kind="ExternalOutput").ap()

    with tile.TileContext(nc) as tc:
        with (
            tc.tile_pool(name="singles", bufs=1) as singles,
            tc.tile_pool(name="ptiles", bufs=8) as ptiles,
            tc.tile_pool(name="recp", bufs=2) as recp,
            tc.tile_pool(name="gscp", bufs=2) as gscp,
            tc.tile_pool(name="ps_big", bufs=3, space="PSUM") as ps_big,
            tc.tile_pool(name="ps_sp", bufs=1, space="PSUM") as ps_sp,
            tc.tile_pool(name="ps_pv", bufs=2, space="PSUM") as ps_pv,
            tc.tile_pool(name="ps_dn", bufs=1, space="PSUM") as ps_dn,
        ):
            # ---- persistent SBUF tiles ----
            xeT = singles.tile([128, 4, EXT], f16)      # [p, e_chunk, seq]
            wq_sb = singles.tile([128, 4, 4, 128], f16)  # [p, j, ke, f]
            wk_sb = singles.tile([128, 4, 4, 128], f16)
            wv_sb = singles.tile([128, 4, E], f16)       # [p, ke, (h f)]
            w1_sb = singles.tile([128, 4, 4, 128], f16)  # [p, j, u, f1]
            w2_sb = singles.tile([128, 4, 4, 128], f16)  # [p, u, ec, e]
            b1_sb = singles.tile([128, 4], f32)
            b2_sb = singles.tile([128, 4], f32)
            qT = singles.tile([128, 4, SC], f16)         # [f_pair, j, q]
            kT = singles.tile([128, 4, EXT], f16)        # [f_pair, j, key]
            v_sb = singles.tile([128, 4, H, FEAT], bf16)  # [key_chunk, sc, h, f]
            x1T = singles.tile([128, 4, SC], f16)        # [f_pair, j, q]
            f_sbT = singles.tile([128, 4, SC], f16)      # [f1_chunk, u, q]
            out_sbT = singles.tile([128, 4, SC], f32)    # [e_chunk, ec, q]
            ones_sb = singles.tile([128, 128], bf16)
            wu_sb = singles.tile([128, 512], f16)

            # ---- input DMAs, in consumption order ----
            nc.sync.dma_start(wq_sb[:, 0], wq_d[0])
            nc.sync.dma_start(wk_sb[:, 0], wk_d[0])
            for ke in range(4):
                nc.sync.dma_start(xeT[:, ke, :], xeT_d[:, ke, :])
            nc.sync.dma_start(wv_sb[:], wv_d[:])
            for j in range(1, 4):
                nc.sync.dma_start(wq_sb[:, j], wq_d[j])
                nc.sync.dma_start(wk_sb[:, j], wk_d[j])
            for j in range(4):
                nc.sync.dma_start(w1_sb[:, j], w1_d[j])
            nc.sync.dma_start(w2_sb[:], w2_d[:])
            nc.sync.dma_start(b1_sb[:], b1_d[:])
            nc.sync.dma_start(b2_sb[:], b2_d[:])

            # ---- constants ----
            nc.gpsimd.memset(wu_sb[:], 0.0)
            nc.gpsimd.memset(ones_sb[:], 1.0)

            # ---- PE warm-up: matmuls on zeros, results never read.
            # The PE only reaches 2.4GHz after ~3us of *continuous*
            # execution; these bridge the input-DMA wait so real matmuls
            # start (and stay) at full clock.
            for _ in range(7):
                wu_ps = ps_big.tile([128, 512], f32, tag="big", name="wu_ps")
                nc.tensor.matmul(wu_ps[:], wu_sb[:, 0:128], wu_sb[:],
                                 start=True, stop=True)

            def emit_proj(j):
                # qT/kT feature chunk of head pair j (heads 2j, 2j+1)
                qp = ps_big.tile([128, SC], f32, tag="big", name="qp")
                for ke in range(4):
                    nc.tensor.matmul(
                        qp[:], wq_sb[:, j, ke, :], xeT[:, ke, 128:384],
                        start=(ke == 0), stop=(ke == 3))
                nc.vector.tensor_copy(out=qT[:, j, :], in_=qp[:])
                kp = ps_big.tile([128, EXT], f32, tag="big", name="kp")
                for ke in range(4):
                    nc.tensor.matmul(
                        kp[:], wk_sb[:, j, ke, :], xeT[:, ke, :],
                        start=(ke == 0), stop=(ke == 3))
                nc.vector.tensor_copy(out=kT[:, j, :], in_=kp[:])

            def emit_v():
                for sc in range(4):
                    vp = ps_big.tile([128, E], f32, tag="big", name="vp")
                    for ke in range(4):
                        nc.tensor.matmul(
                            vp[:], xeT[:, ke, 128 * sc:128 * sc + 128],
                            wv_sb[:, ke, :],
                            start=(ke == 0), stop=(ke == 3))
                    nc.scalar.copy(
                        out=v_sb[:, sc, :, :],
                        in_=vp[:].rearrange("p (h f) -> p h f", h=H))

            # per (ca): valid query sub-blocks and the triangular mask.
            #   ca0: s0 triangular(kc>=q), s1 empty
            #   ca1: s0 full, s1 triangular(kc>=q)
            #   ca2: s0 triangular(kc<=q), s1 full
            #   ca3: s0 empty, s1 triangular(kc<=q)
            QSL = [(0, 128), (0, 256), (0, 256), (128, 256)]
            TRI = [(0, 1), (1, 1), (0, -1), (1, -1)]  # (s_tri, sign)

            def emit_scores(j):
                # the two heads' scores run row-tiled (K=64 at row groups 0
                # and 64) but must land in separate PSUM banks: two matmul
                # groups writing one bank fault the PE.
                p_list = []
                for ca in range(4):
                    lo, hi = QSL[ca]
                    ns = (hi - lo) // 128
                    s0 = lo // 128
                    p_t = ptiles.tile([128, 2, 2, 128], bf16, tag="p",
                                      name="p_t")  # [key, s, hh, q]
                    sps = []
                    for hh in range(2):
                        o = 64 * hh
                        sp = ps_sp.tile([128, SC], f32, tag=f"sp{hh}",
                                        name=f"sp{hh}")
                        nc.tensor.matmul(
                            sp[:, lo:hi],
                            kT[o:o + 64, j, 128 * ca:128 * ca + 128],
                            qT[o:o + 64, j, lo:hi],
                            start=True, stop=True)
                        sps.append(sp)
                    for hh in range(2):
                        nc.scalar.activation(
                            out=p_t[:, s0:s0 + ns, hh, :],
                            in_=sps[hh][:, lo:hi].rearrange(
                                "p (s q) -> p s q", s=ns),
                            func=Act.Exp)
                    s_tri, sign = TRI[ca]
                    nc.gpsimd.affine_select(
                        out=p_t[:, s_tri, :, :], in_=p_t[:, s_tri, :, :],
                        compare_op=Alu.is_ge, fill=0.0, base=0,
                        channel_multiplier=sign,
                        pattern=[[0, 2], [-sign, 128]])
                    p_list.append(p_t)
                return p_list

            pvps = {}
            dns = {}

            def emit_pv_dn(j, p_list):
                # p@v: one bank, even head rows 0:64, odd rows 64:128.
                # ca order starts with ca1 (full-width) so the start=True
                # matmul covers the whole accumulation region.
                pv_ps = ps_pv.tile([128, SC], f32, tag="pv", name="pv_ps")
                for hh in range(2):
                    h = 2 * j + hh
                    rows = slice(64 * hh, 64 * hh + 64)
                    for i, ca in enumerate((1, 0, 2, 3)):
                        lo, hi = QSL[ca]
                        ns = (hi - lo) // 128
                        s0 = lo // 128
                        nc.tensor.matmul(
                            pv_ps[rows, lo:hi],
                            v_sb[:, ca, h, :],
                            p_list[ca][:, s0:s0 + ns, hh, :],
                            start=(i == 0), stop=(i == 3))
                # denominators for both heads, replicated to all 128
                # partitions by an all-ones stationary operand (no
                # partition broadcast needed): [128, (s, hh, q)]
                dn_ps = ps_dn.tile([128, 512], f32, tag="dn", name="dn_ps")
                for i, ca in enumerate((1, 0, 2, 3)):
                    lo, hi = QSL[ca]
                    ns = (hi - lo) // 128
                    s0 = lo // 128
                    nc.tensor.matmul(
                        dn_ps[:, 256 * s0:256 * s0 + 256 * ns],
                        ones_sb[:],
                        p_list[ca][:, s0:s0 + ns, :, :],
                        start=(i == 0), stop=(i == 3))
                pvps[j] = pv_ps
                dns[j] = dn_ps

            def emit_norm(j):
                dn_ps = dns.pop(j)
                pv_ps = pvps.pop(j)
                rec = recp.tile([128, 512], f32, tag="rec", name="rec")
                nc.vector.reciprocal_approx_fast(out=rec[:], in_=dn_ps[:])
                bv = rec[:].rearrange("p (s h q) -> p s h q", s=2, h=2)
                for hh in range(2):
                    rows = slice(64 * hh, 64 * hh + 64)
                    nc.vector.tensor_mul(
                        out=x1T[rows, j, :].rearrange("p (s q) -> p s q", s=2),
                        in0=pv_ps[rows, :].rearrange("p (s q) -> p s q", s=2),
                        in1=bv[rows, :, hh, :])

            # ---- attention, software pipelined ----
            emit_proj(0)
            emit_proj(1)
            scores = [emit_scores(0)]
            emit_v()
            emit_proj(2)
            scores.append(emit_scores(1))
            emit_pv_dn(0, scores[0])
            emit_proj(3)
            scores.append(emit_scores(2))
            emit_norm(0)
            emit_pv_dn(1, scores[1])
            scores.append(emit_scores(3))
            emit_norm(1)
            emit_pv_dn(2, scores[2])
            emit_norm(2)
            emit_pv_dn(3, scores[3])
            emit_norm(3)

            # ---- FFN1 (transposed): f^T[u] = relu(W1_u^T x1 + b1_u) ----
            for u in range(4):
                f1_ps = ps_big.tile([128, SC], f32, tag="big",
                                    name=f"f1_ps{u}")
                for j in range(4):
                    nc.tensor.matmul(
                        f1_ps[:], w1_sb[:, j, u, :], x1T[:, j, :],
                        start=(j == 0), stop=(j == 3))
                nc.scalar.activation(
                    out=f_sbT[:, u, :], in_=f1_ps[:], func=Act.Relu,
                    bias=b1_sb[:, u:u + 1], scale=1.0)

            # ---- FFN2 (transposed) + relu + residual + out DMA ----
            for ec in range(4):
                gp = ps_big.tile([128, SC], f32, tag="big", name="gp")
                for u in range(4):
                    nc.tensor.matmul(
                        gp[:], w2_sb[:, u, ec, :], f_sbT[:, u, :],
                        start=(u == 0), stop=(u == 3))
                gsc = gscp.tile([128, SC], f32, tag="gsc", name="gsc")
                nc.scalar.activation(
                    out=gsc[:], in_=gp[:], func=Act.Relu,
                    bias=b2_sb[:, ec:ec + 1], scale=1.0)
                nc.vector.tensor_add(
                    out=out_sbT[:, ec, :], in0=gsc[:],
                    in1=xeT[:, ec, 128:384])
                nc.sync.dma_start(out_d[ec], out_sbT[:, ec, :])

    nc.compile()
    return nc


def _get_nc():
    if "nc" not in _CACHE:
        _CACHE["nc"] = _build_nc()
    return _CACHE["nc"]


def _pack_weights(Wq, Wk, Wv, W1, b1, W2, b2):
    f16 = np.float16
    Wq = np.asarray(Wq, np.float32)
    Wk = np.asarray(Wk, np.float32)
    Wv = np.asarray(Wv, np.float32)
    W1 = np.asarray(W1, np.float32)
    W2 = np.asarray(W2, np.float32)
    # wq[j, p, ke, f] = Wq[128j+f, 128ke+p]
    wq = np.ascontiguousarray(
        Wq.reshape(4, 128, 4, 128).transpose(0, 3, 2, 1).astype(f16))
    wk = np.ascontiguousarray(
        Wk.reshape(4, 128, 4, 128).transpose(0, 3, 2, 1).astype(f16))
    # wv[p, ke, hf] = Wv[hf, 128ke+p]
    wv = np.ascontiguousarray(
        Wv.reshape(512, 4, 128).transpose(2, 1, 0).astype(f16))
    # w1[j, p, u, f1] = (W1/8)[128u+f1, 128j+p]  (1/sqrt(FEAT) folded in)
    w1 = np.ascontiguousarray(
        (W1 / np.sqrt(np.float32(FEAT))).reshape(4, 128, 4, 128)
        .transpose(2, 3, 0, 1).astype(f16))
    # w2[p, u, ec, e] = W2[128ec+e, 128u+p]
    w2 = np.ascontiguousarray(
        W2.reshape(4, 128, 4, 128).transpose(3, 2, 0, 1).astype(f16))
    b1p = np.ascontiguousarray(
        np.asarray(b1, np.float32).reshape(4, 128).T)
    b2p = np.ascontiguousarray(
        np.asarray(b2, np.float32).reshape(4, 128).T)
    return {"wq": wq, "wk": wk, "wv": wv, "w1": w1, "w2": w2,
            "b1": b1p, "b2": b2p}


def _shard_inputs(x, Wq, Wk, Wv, W1, b1, W2, b2):
    x2 = np.ascontiguousarray(np.asarray(x, dtype=np.float32).reshape(S, E))
    ws = _pack_weights(Wq, Wk, Wv, W1, b1, W2, b2)
    in_maps = []
    for c in range(N_CORES):
        r_, half = c >> 1, c & 1
        eidx = np.arange(256 * half - 128, 256 * half + 384)
        valid = (eidx >= 0) & (eidx < S // DIL)
        xe = np.zeros((EXT, E), np.float32)
        xe[valid] = x2[DIL * eidx[valid] + r_]
        # xeT[p, eo, s] = xe[s, 128eo+p]
        xeT = np.ascontiguousarray(
            xe.T.reshape(4, 128, EXT).transpose(1, 0, 2).astype(np.float16))
        in_maps.append({"xeT": xeT, **ws})
    return in_maps


def _gather_outputs(results):
    out = np.zeros((S, E), np.float32)
    for c in range(N_CORES):
        r_, half = c >> 1, c & 1
        # res "out" is [4, 128, SC] = out^T chunks; -> [SC, E]
        oT = np.asarray(results[c]["out"]).reshape(E, SC)
        i = np.arange(256 * half, 256 * half + SC)
        out[DIL * i + r_] = oT.T
    return out.reshape(1, S, E)


def run(inputs, trace=False, tmpdir=None):
    from concourse import bass_utils
    nc = _get_nc()
    in_maps = _shard_inputs(**inputs)
    res = bass_utils.run_bass_kernel_spmd(
        nc, in_maps, list(range(N_CORES)), trace=trace, tmpdir=tmpdir)
    return _gather_outputs(res.results), res


def kernel(x, Wq, Wk, Wv, W1, b1, W2, b2):
    out, _ = run(dict(x=x, Wq=Wq, Wk=Wk, Wv=Wv, W1=W1, b1=b1, W2=W2, b2=b2))
    return out


# revision 21
# speedup vs baseline: 1.0546x; 1.0546x over previous
"""LongFormer dilated-window attention block on 8 trn2 NeuronCores.

Sharding: 8 cores = 4 dilation residues x 2 sequence halves. Query q
attends keys q + 4*j - 512 (j=0..256), i.e. only keys with the same
residue mod DIL=4. De-interleaving by residue turns the dilated window
into a contiguous +-128 sliding window in "residue space". Each core
gets a zero-padded [512, 512] slice of x: its 256 owned rows plus a
128-row halo on each side (all in residue space), so no communication
is needed.

v2 design (vs the fp32 v1 baseline):
- Host pre-casts x and all weights to fp16 and pre-packs them into the
  exact SBUF layouts (x pre-transposed to [e, seq]; weights pre-tiled
  per stationary 128x128 tile). Halves HBM traffic, turns every DMA
  into contiguous >=1KB partition lines, and eliminates all on-chip PE
  transposes and the identity constant.
- All matmuls run fp16 (bf16 for softmax probabilities, which can
  reach exp(44) and overflow fp16). fp16/bf16 weights enable fast
  weight load; fp32 weights do not.
- Scores/p@v are trimmed to the 3 valid 128-key diagonal blocks per
  128-query block (the +-128 window spans 384 keys, not 512).
- Both heads of a pair pack into one PSUM bank: scores as one
  2-row-group matmul group [keys, 2, q]; p@v as even rows 0:64 /
  odd rows 64:128 (col-grp tile_position).
- Softmax denominators via ones-vector matmuls into a [1, 512] bank;
  normalization = DVE reciprocal -> gpsimd partition_broadcast -> DVE
  muls (no PE broadcast/shift matmuls). 1/sqrt(d) is folded into W1
  host-side (FFN1 is linear in x1).
- FFN runs fully transposed ([f, q] layouts) so FFN1's output feeds
  FFN2 directly with no transposes; output is written transposed and
  the host transposes it back. Residual comes from the x^T tile.
- A few warm-up matmuls on a zeroed tile run during the input DMAs to
  lift the PE out of its cold 1.2GHz HAM state before real work.
"""

import sys

if "/opt/trn_rl_repo" not in sys.path:
    sys.path.insert(0, "/opt/trn_rl_repo")

import numpy as np

N_CORES = 8
S, E, H, FEAT = 2048, 512, 8, 64
DIL = 4
SC = 256      # owned queries per core (residue space)
EXT = 512     # ext rows per core (owned + 128 halo each side)

_CACHE = {}


def _build_nc():
    import concourse.bacc as bacc
    import concourse.tile as tile
    import concourse.mybir as mybir
    import concourse.bass as bass

    dt = mybir.dt
    f32 = dt.float32
    f16 = dt.float16
    bf16 = dt.bfloat16
    Alu = mybir.AluOpType
    Act = mybir.ActivationFunctionType

    nc = bacc.Bacc("TRN2", target_bir_lowering=False, debug=False,
                   num_devices=N_CORES)

    # ---- DRAM I/O (host pre-packed, fp16) ----
    xeT_d = nc.dram_tensor("xeT", [128, 4, EXT], f16, kind="ExternalInput").ap()
    wq_d = nc.dram_tensor("wq", [4, 128, 4, 128], f16, kind="ExternalInput").ap()
    wk_d = nc.dram_tensor("wk", [4, 128, 4, 128], f16, kind="ExternalInput").ap()
    wv_d = nc.dram_tensor("wv", [128, 4, E], f16, kind="ExternalInput").ap()
    w1_d = nc.dram_tensor("w1", [4, 128, 4, 128], f16, kind="ExternalInput").ap()
    w2_d = nc.dram_tensor("w2", [128, 4, 4, 128], f16, kind="ExternalInput").ap()
    b1_d = nc.dram_tensor("b1", [128, 4], f32, kind="ExternalInput").ap()
    b2_d = nc.dram_tensor("b2", [128, 4], f32, kind="ExternalInput").ap()
    out_d = nc.dram_tensor("out", [4, 128, SC], f32, kind="ExternalOutput").ap()

    with tile.TileContext(nc) as tc:
        with (
            tc.tile_pool(name="singles", bufs=1) as singles,
            tc.tile_pool(name="ptiles", bufs=8) as ptiles,
            tc.tile_pool(name="recp", bufs=2) as recp,
            tc.tile_pool(name="gscp", bufs=2) as gscp,
            tc.tile_pool(name="ps_big", bufs=3, space="PSUM") as ps_big,
            tc.tile_pool(name="ps_sp", bufs=1, space="PSUM") as ps_sp,
            tc.tile_pool(name="ps_pv", bufs=2, space="PSUM") as ps_pv,
            tc.tile_pool(name="ps_dn", bufs=1, space="PSUM") as ps_dn,
        ):
            # ---- persistent SBUF tiles ----
            # xeT as 4 independent tiles so each DMA chunk unblocks its
            # consumers without waiting for the whole tensor.
            xeT = [singles.tile([128, EXT], f16, name=f"xeT{ke}")
                   for ke in range(4)]
            wq_sb = singles.tile([128, 4, 4, 128], f16)  # [p, j, ke, f]
            wk_sb = singles.tile([128, 4, 4, 128], f16)
            wv_sb = singles.tile([128, 4, E], f16)       # [p, ke, (h f)]
            w1_sb = singles.tile([128, 4, 4, 128], f16)  # [p, j, u, f1]
            w2_sb = singles.tile([128, 4, 4, 128], f16)  # [p, u, ec, e]
            b1_sb = singles.tile([128, 4], f32)
            b2_sb = singles.tile([128, 4], f32)
            qT = singles.tile([128, 4, SC], f16)         # [f_pair, j, q]
            kT = singles.tile([128, 4, EXT], f16)        # [f_pair, j, key]
            v_sb = singles.tile([128, 4, H, FEAT], bf16)  # [key_chunk, sc, h, f]
            x1T = singles.tile([128, 4, SC], f16)        # [f_pair, j, q]
            f_sbT = singles.tile([128, 4, SC], f16)      # [f1_chunk, u, q]
            out_sbT = singles.tile([128, 4, SC], f32)    # [e_chunk, ec, q]
            ones_sb = singles.tile([128, 128], bf16)
            wu_sb = singles.tile([128, 512], f16)

            # ---- input DMAs, in consumption order ----
            nc.sync.dma_start(wq_sb[:, 0], wq_d[0])
            nc.sync.dma_start(wk_sb[:, 0], wk_d[0])
            for ke in range(4):
                nc.sync.dma_start(xeT[ke][:], xeT_d[:, ke, :])
            for j in range(1, 4):
                nc.sync.dma_start(wq_sb[:, j], wq_d[j])
                nc.sync.dma_start(wk_sb[:, j], wk_d[j])
            nc.sync.dma_start(wv_sb[:], wv_d[:])
            for j in range(4):
                nc.sync.dma_start(w1_sb[:, j], w1_d[j])
            nc.sync.dma_start(w2_sb[:], w2_d[:])
            nc.sync.dma_start(b1_sb[:], b1_d[:])
            nc.sync.dma_start(b2_sb[:], b2_d[:])

            # ---- constants ----
            nc.gpsimd.memset(wu_sb[:], 0.0)
            nc.gpsimd.memset(ones_sb[:], 1.0)

            # ---- PE warm-up: matmuls on zeros, results never read.
            # The PE only reaches 2.4GHz after ~3us of *continuous*
            # execution; these bridge the input-DMA wait so real matmuls
            # start (and stay) at full clock.
            for _ in range(7):
                wu_ps = ps_big.tile([128, 512], f32, tag="big", name="wu_ps")
                nc.tensor.matmul(wu_ps[:], wu_sb[:, 0:128], wu_sb[:],
                                 start=True, stop=True)

            def emit_proj(j):
                # qT/kT feature chunk of head pair j (heads 2j, 2j+1)
                qp = ps_big.tile([128, SC], f32, tag="big", name="qp")
                for ke in range(4):
                    nc.tensor.matmul(
                        qp[:], wq_sb[:, j, ke, :], xeT[ke][:, 128:384],
                        start=(ke == 0), stop=(ke == 3))
                nc.vector.tensor_copy(out=qT[:, j, :], in_=qp[:])
                kp = ps_big.tile([128, EXT], f32, tag="big", name="kp")
                for ke in range(4):
                    nc.tensor.matmul(
                        kp[:], wk_sb[:, j, ke, :], xeT[ke][:],
                        start=(ke == 0), stop=(ke == 3))
                # halves, so scores ca0/ca1 gate only on the first 256 keys
                nc.vector.tensor_copy(out=kT[:, j, 0:256], in_=kp[:, 0:256])
                nc.vector.tensor_copy(out=kT[:, j, 256:512],
                                      in_=kp[:, 256:512])

            def emit_v():
                for sc in range(4):
                    vp = ps_big.tile([128, E], f32, tag="big", name="vp")
                    for ke in range(4):
                        nc.tensor.matmul(
                            vp[:], xeT[ke][:, 128 * sc:128 * sc + 128],
                            wv_sb[:, ke, :],
                            start=(ke == 0), stop=(ke == 3))
                    nc.scalar.copy(
                        out=v_sb[:, sc, :, :],
                        in_=vp[:].rearrange("p (h f) -> p h f", h=H))

            # per (ca): valid query sub-blocks and the triangular mask.
            #   ca0: s0 triangular(kc>=q), s1 empty
            #   ca1: s0 full, s1 triangular(kc>=q)
            #   ca2: s0 triangular(kc<=q), s1 full
            #   ca3: s0 empty, s1 triangular(kc<=q)
            QSL = [(0, 128), (0, 256), (0, 256), (128, 256)]
            TRI = [(0, 1), (1, 1), (0, -1), (1, -1)]  # (s_tri, sign)

            def emit_scores(j):
                # the two heads' scores run row-tiled (K=64 at row groups 0
                # and 64) but must land in separate PSUM banks: two matmul
                # groups writing one bank fault the PE. One padded sp tile
                # spans two banks (hh stride = full bank) so a single exp
                # covers both heads, halving the ACT per-op overhead.
                p_list = []
                for ca in range(4):
                    lo, hi = QSL[ca]
                    ns = (hi - lo) // 128
                    s0 = lo // 128
                    p_t = ptiles.tile([128, 2, 2, 128], bf16, tag="p",
                                      name="p_t")  # [key, s, hh, q]
                    sp = ps_sp.tile([128, 2, SC], f32, tag="sp", name="sp",
                                    padded_shape=[128, 2, 512])
                    for hh in range(2):
                        o = 64 * hh
                        nc.tensor.matmul(
                            sp[:, hh, lo:hi],
                            kT[o:o + 64, j, 128 * ca:128 * ca + 128],
                            qT[o:o + 64, j, lo:hi],
                            start=True, stop=True)
                    nc.scalar.activation(
                        out=p_t[:, s0:s0 + ns, :, :].rearrange(
                            "p s h q -> p h s q"),
                        in_=sp[:, :, lo:hi].rearrange(
                            "p h (s q) -> p h s q", s=ns),
                        func=Act.Exp)
                    s_tri, sign = TRI[ca]
                    nc.gpsimd.affine_select(
                        out=p_t[:, s_tri, :, :], in_=p_t[:, s_tri, :, :],
                        compare_op=Alu.is_ge, fill=0.0, base=0,
                        channel_multiplier=sign,
                        pattern=[[0, 2], [-sign, 128]])
                    p_list.append(p_t)
                return p_list

            pvps = {}
            dns = {}

            def emit_pv_dn(j, p_list):
                # p@v: one bank, even head rows 0:64, odd rows 64:128.
                # ca order starts with ca1 (full-width) so the start=True
                # matmul covers the whole accumulation region.
                pv_ps = ps_pv.tile([128, SC], f32, tag="pv", name="pv_ps")
                for hh in range(2):
                    h = 2 * j + hh
                    rows = slice(64 * hh, 64 * hh + 64)
                    for i, ca in enumerate((1, 0, 2, 3)):
                        lo, hi = QSL[ca]
                        ns = (hi - lo) // 128
                        s0 = lo // 128
                        nc.tensor.matmul(
                            pv_ps[rows, lo:hi],
                            v_sb[:, ca, h, :],
                            p_list[ca][:, s0:s0 + ns, hh, :],
                            start=(i == 0), stop=(i == 3))
                # denominators for both heads, replicated to all 128
                # partitions by an all-ones stationary operand (no
                # partition broadcast needed): [128, (s, hh, q)]
                dn_ps = ps_dn.tile([128, 512], f32, tag="dn", name="dn_ps")
                for i, ca in enumerate((1, 0, 2, 3)):
                    lo, hi = QSL[ca]
                    ns = (hi - lo) // 128
                    s0 = lo // 128
                    nc.tensor.matmul(
                        dn_ps[:, 256 * s0:256 * s0 + 256 * ns],
                        ones_sb[:],
                        p_list[ca][:, s0:s0 + ns, :, :],
                        start=(i == 0), stop=(i == 3))
                pvps[j] = pv_ps
                dns[j] = dn_ps

            def emit_norm(j):
                dn_ps = dns.pop(j)
                pv_ps = pvps.pop(j)
                rec = recp.tile([128, 512], f32, tag="rec", name="rec")
                nc.vector.reciprocal_approx_fast(out=rec[:], in_=dn_ps[:])
                bv = rec[:].rearrange("p (s h q) -> p s h q", s=2, h=2)
                for hh in range(2):
                    rows = slice(64 * hh, 64 * hh + 64)
                    nc.vector.tensor_mul(
                        out=x1T[rows, j, :].rearrange("p (s q) -> p s q", s=2),
                        in0=pv_ps[rows, :].rearrange("p (s q) -> p s q", s=2),
                        in1=bv[rows, :, hh, :])

            # ---- attention, software pipelined ----
            emit_proj(0)
            emit_proj(1)
            scores = [emit_scores(0)]
            emit_v()
            emit_proj(2)
            scores.append(emit_scores(1))
            emit_pv_dn(0, scores[0])
            emit_proj(3)
            scores.append(emit_scores(2))
            emit_norm(0)
            emit_pv_dn(1, scores[1])
            scores.append(emit_scores(3))
            emit_norm(1)
            emit_pv_dn(2, scores[2])
            emit_norm(2)
            emit_pv_dn(3, scores[3])
            emit_norm(3)

            # ---- FFN1 (transposed): f^T[u] = relu(W1_u^T x1 + b1_u) ----
            for u in range(4):
                f1_ps = ps_big.tile([128, SC], f32, tag="big",
                                    name=f"f1_ps{u}")
                for j in range(4):
                    nc.tensor.matmul(
                        f1_ps[:], w1_sb[:, j, u, :], x1T[:, j, :],
                        start=(j == 0), stop=(j == 3))
                nc.scalar.activation(
                    out=f_sbT[:, u, :], in_=f1_ps[:], func=Act.Relu,
                    bias=b1_sb[:, u:u + 1], scale=1.0)

            # ---- FFN2 (transposed) + relu + residual + out DMA ----
            for ec in range(4):
                gp = ps_big.tile([128, SC], f32, tag="big", name="gp")
                for u in range(4):
                    nc.tensor.matmul(
                        gp[:], w2_sb[:, u, ec, :], f_sbT[:, u, :],
                        start=(u == 0), stop=(u == 3))
                gsc = gscp.tile([128, SC], f32, tag="gsc", name="gsc")
                nc.scalar.activation(
                    out=gsc[:], in_=gp[:], func=Act.Relu,
                    bias=b2_sb[:, ec:ec + 1], scale=1.0)
                nc.vector.tensor_add(
                    out=out_sbT[:, ec, :], in0=gsc[:],
                    in1=xeT[ec][:, 128:384])
                nc.sync.dma_start(out_d[ec], out_sbT[:, ec, :])

    nc.compile()
    return nc


def _get_nc():
    if "nc" not in _CACHE:
        _CACHE["nc"] = _build_nc()
    return _CACHE["nc"]


def _pack_weights(Wq, Wk, Wv, W1, b1, W2, b2):
    f16 = np.float16
    Wq = np.asarray(Wq, np.float32)
    Wk = np.asarray(Wk, np.float32)
    Wv = np.asarray(Wv, np.float32)
    W1 = np.asarray(W1, np.float32)
    W2 = np.asarray(W2, np.float32)
    # wq[j, p, ke, f] = Wq[128j+f, 128ke+p]
    wq = np.ascontiguousarray(
        Wq.reshape(4, 128, 4, 128).transpose(0, 3, 2, 1).astype(f16))
    wk = np.ascontiguousarray(
        Wk.reshape(4, 128, 4, 128).transpose(0, 3, 2, 1).astype(f16))
    # wv[p, ke, hf] = Wv[hf, 128ke+p]
    wv = np.ascontiguousarray(
        Wv.reshape(512, 4, 128).transpose(2, 1, 0).astype(f16))
    # w1[j, p, u, f1] = (W1/8)[128u+f1, 128j+p]  (1/sqrt(FEAT) folded in)
    w1 = np.ascontiguousarray(
        (W1 / np.sqrt(np.float32(FEAT))).reshape(4, 128, 4, 128)
        .transpose(2, 3, 0, 1).astype(f16))
    # w2[p, u, ec, e] = W2[128ec+e, 128u+p]
    w2 = np.ascontiguousarray(
        W2.reshape(4, 128, 4, 128).transpose(3, 2, 0, 1).astype(f16))
    b1p = np.ascontiguousarray(
        np.asarray(b1, np.float32).reshape(4, 128).T)
    b2p = np.ascontiguousarray(
        np.asarray(b2, np.float32).reshape(4, 128).T)
    return {"wq": wq, "wk": wk, "wv": wv, "w1": w1, "w2": w2,
            "b1": b1p, "b2": b2p}


def _shard_inputs(x, Wq, Wk, Wv, W1, b1, W2, b2):
    x2 = np.ascontiguousarray(np.asarray(x, dtype=np.float32).reshape(S, E))
    ws = _pack_weights(Wq, Wk, Wv, W1, b1, W2, b2)
    in_maps = []
    for c in range(N_CORES):
        r_, half = c >> 1, c & 1
        eidx = np.arange(256 * half - 128, 256 * half + 384)
        valid = (eidx >= 0) & (eidx < S // DIL)
        xe = np.zeros((EXT, E), np.float32)
        xe[valid] = x2[DIL * eidx[valid] + r_]
        # xeT[p, eo, s] = xe[s, 128eo+p]
        xeT = np.ascontiguousarray(
            xe.T.reshape(4, 128, EXT).transpose(1, 0, 2).astype(np.float16))
        in_maps.append({"xeT": xeT, **ws})
    return in_maps


def _gather_outputs(results):
    out = np.zeros((S, E), np.float32)
    for c in range(N_CORES):
        r_, half = c >> 1, c & 1
        # res "out" is [4, 128, SC] = out^T chunks; -> [SC, E]
        oT = np.asarray(results[c]["out"]).reshape(E, SC)
        i = np.arange(256 * half, 256 * half + SC)
        out[DIL * i + r_] = oT.T
    return out.reshape(1, S, E)


def run(inputs, trace=False, tmpdir=None):
    from concourse import bass_utils
    nc = _get_nc()
    in_maps = _shard_inputs(**inputs)
    res = bass_utils.run_bass_kernel_spmd(
        nc, in_maps, list(range(N_CORES)), trace=trace, tmpdir=tmpdir)
    return _gather_outputs(res.results), res


def kernel(x, Wq, Wk, Wv, W1, b1, W2, b2):
    out, _ = run(dict(x=x, Wq=Wq, Wk=Wk, Wv=Wv, W1=W1, b1=b1, W2=W2, b2=b2))
    return out


# revision 27
# speedup vs baseline: 1.0618x; 1.0068x over previous
"""LongFormer dilated-window attention block on 8 trn2 NeuronCores.

Sharding: 8 cores = 4 dilation residues x 2 sequence halves. Query q
attends keys q + 4*j - 512 (j=0..256), i.e. only keys with the same
residue mod DIL=4. De-interleaving by residue turns the dilated window
into a contiguous +-128 sliding window in "residue space". Each core
gets a zero-padded [512, 512] slice of x: its 256 owned rows plus a
128-row halo on each side (all in residue space), so no communication
is needed.

v2 design (vs the fp32 v1 baseline):
- Host pre-casts x and all weights to fp16 and pre-packs them into the
  exact SBUF layouts (x pre-transposed to [e, seq]; weights pre-tiled
  per stationary 128x128 tile). Halves HBM traffic, turns every DMA
  into contiguous >=1KB partition lines, and eliminates all on-chip PE
  transposes and the identity constant.
- All matmuls run fp16 (bf16 for softmax probabilities, which can
  reach exp(44) and overflow fp16). fp16/bf16 weights enable fast
  weight load; fp32 weights do not.
- Scores/p@v are trimmed to the 3 valid 128-key diagonal blocks per
  128-query block (the +-128 window spans 384 keys, not 512).
- Both heads of a pair pack into one PSUM bank: scores as one
  2-row-group matmul group [keys, 2, q]; p@v as even rows 0:64 /
  odd rows 64:128 (col-grp tile_position).
- Softmax denominators via ones-vector matmuls into a [1, 512] bank;
  normalization = DVE reciprocal -> gpsimd partition_broadcast -> DVE
  muls (no PE broadcast/shift matmuls). 1/sqrt(d) is folded into W1
  host-side (FFN1 is linear in x1).
- FFN runs fully transposed ([f, q] layouts) so FFN1's output feeds
  FFN2 directly with no transposes; output is written transposed and
  the host transposes it back. Residual comes from the x^T tile.
- A few warm-up matmuls on a zeroed tile run during the input DMAs to
  lift the PE out of its cold 1.2GHz HAM state before real work.
"""

import sys

if "/opt/trn_rl_repo" not in sys.path:
    sys.path.insert(0, "/opt/trn_rl_repo")

import numpy as np

N_CORES = 8
S, E, H, FEAT = 2048, 512, 8, 64
DIL = 4
SC = 256      # owned queries per core (residue space)
EXT = 512     # ext rows per core (owned + 128 halo each side)

_CACHE = {}


def _build_nc():
    import concourse.bacc as bacc
    import concourse.tile as tile
    import concourse.mybir as mybir
    import concourse.bass as bass

    dt = mybir.dt
    f32 = dt.float32
    f16 = dt.float16
    bf16 = dt.bfloat16
    Alu = mybir.AluOpType
    Act = mybir.ActivationFunctionType

    nc = bacc.Bacc("TRN2", target_bir_lowering=False, debug=False,
                   num_devices=N_CORES)

    # ---- DRAM I/O (host pre-packed, fp16) ----
    xeT_d = nc.dram_tensor("xeT", [128, 4, EXT], f16, kind="ExternalInput").ap()
    wq_d = nc.dram_tensor("wq", [4, 128, 4, 128], f16, kind="ExternalInput").ap()
    wk_d = nc.dram_tensor("wk", [4, 128, 4, 128], f16, kind="ExternalInput").ap()
    wv_d = nc.dram_tensor("wv", [128, 4, E], f16, kind="ExternalInput").ap()
    w1_d = nc.dram_tensor("w1", [4, 128, 4, 128], f16, kind="ExternalInput").ap()
    w2_d = nc.dram_tensor("w2", [128, 4, 4, 128], f16, kind="ExternalInput").ap()
    b1_d = nc.dram_tensor("b1", [128, 4], f32, kind="ExternalInput").ap()
    b2_d = nc.dram_tensor("b2", [128, 4], f32, kind="ExternalInput").ap()
    out_d = nc.dram_tensor("out", [4, 128, SC], f16, kind="ExternalOutput").ap()

    with tile.TileContext(nc) as tc:
        with (
            tc.tile_pool(name="singles", bufs=1) as singles,
            tc.tile_pool(name="ptiles", bufs=8) as ptiles,
            tc.tile_pool(name="recp", bufs=2) as recp,
            tc.tile_pool(name="gscp", bufs=2) as gscp,
            tc.tile_pool(name="ps_big", bufs=3, space="PSUM") as ps_big,
            tc.tile_pool(name="ps_sp", bufs=1, space="PSUM") as ps_sp,
            tc.tile_pool(name="ps_pv", bufs=2, space="PSUM") as ps_pv,
            tc.tile_pool(name="ps_dn", bufs=1, space="PSUM") as ps_dn,
        ):
            # ---- persistent SBUF tiles ----
            # xeT as 4 independent tiles so each DMA chunk unblocks its
            # consumers without waiting for the whole tensor.
            xeT = [singles.tile([128, EXT], f16, name=f"xeT{ke}")
                   for ke in range(4)]
            wq_sb = singles.tile([128, 4, 4, 128], f16)  # [p, j, ke, f]
            wk_sb = singles.tile([128, 4, 4, 128], f16)
            wv_sb = singles.tile([128, 4, E], f16)       # [p, ke, (h f)]
            w1_sb = singles.tile([128, 4, 4, 128], f16)  # [p, j, u, f1]
            w2_sb = singles.tile([128, 4, 4, 128], f16)  # [p, u, ec, e]
            b1_sb = singles.tile([128, 4], f32)
            b2_sb = singles.tile([128, 4], f32)
            qT = singles.tile([128, 4, SC], f16)         # [f_pair, j, q]
            kT = singles.tile([128, 4, EXT], f16)        # [f_pair, j, key]
            v_sb = singles.tile([128, 4, H, FEAT], bf16)  # [key_chunk, sc, h, f]
            x1T = singles.tile([128, 4, SC], f16)        # [f_pair, j, q]
            f_sbT = singles.tile([128, 4, SC], f16)      # [f1_chunk, u, q]
            out_sbT = singles.tile([128, 4, SC], f16)    # [e_chunk, ec, q]
            ones_sb = singles.tile([128, 128], bf16)
            wu_sb = singles.tile([128, 512], f16)

            # ---- input DMAs, in consumption order ----
            nc.sync.dma_start(wq_sb[:, 0], wq_d[0])
            nc.sync.dma_start(xeT[0][:], xeT_d[:, 0, :])
            nc.sync.dma_start(wk_sb[:, 0], wk_d[0])
            for ke in range(1, 4):
                nc.sync.dma_start(xeT[ke][:], xeT_d[:, ke, :])
            nc.sync.dma_start(wq_sb[:, 1:4], wq_d[1:4].rearrange(
                "j p k f -> p j k f"))
            nc.sync.dma_start(wk_sb[:, 1:4], wk_d[1:4].rearrange(
                "j p k f -> p j k f"))
            nc.sync.dma_start(wv_sb[:], wv_d[:])
            nc.sync.dma_start(w1_sb[:], w1_d[:].rearrange(
                "j p k f -> p j k f"))
            nc.sync.dma_start(w2_sb[:], w2_d[:])
            nc.sync.dma_start(b1_sb[:], b1_d[:])
            nc.sync.dma_start(b2_sb[:], b2_d[:])

            # ---- constants ----
            nc.gpsimd.memset(wu_sb[:], 0.0)
            nc.gpsimd.memset(ones_sb[:], 1.0)

            # ---- PE warm-up: matmuls on zeros, results never read.
            # The PE only reaches 2.4GHz after ~3us of *continuous*
            # execution; these bridge the input-DMA wait so real matmuls
            # start (and stay) at full clock.
            def emit_dummy(n):
                for _ in range(n):
                    wu_ps = ps_big.tile([128, 512], f32, tag="big",
                                        name="wu_ps")
                    nc.tensor.matmul(wu_ps[:], wu_sb[:, 0:128], wu_sb[:],
                                     start=True, stop=True)

            emit_dummy(8)

            def emit_proj(j):
                # qT/kT feature chunk of head pair j (heads 2j, 2j+1)
                qp = ps_big.tile([128, SC], f32, tag="big", name="qp")
                for ke in range(4):
                    nc.tensor.matmul(
                        qp[:], wq_sb[:, j, ke, :], xeT[ke][:, 128:384],
                        start=(ke == 0), stop=(ke == 3))
                nc.vector.tensor_copy(out=qT[:, j, :], in_=qp[:])
                kp = ps_big.tile([128, EXT], f32, tag="big", name="kp")
                for ke in range(4):
                    nc.tensor.matmul(
                        kp[:], wk_sb[:, j, ke, :], xeT[ke][:],
                        start=(ke == 0), stop=(ke == 3))
                # halves, so scores ca0/ca1 gate only on the first 256 keys
                nc.vector.tensor_copy(out=kT[:, j, 0:256], in_=kp[:, 0:256])
                nc.vector.tensor_copy(out=kT[:, j, 256:512],
                                      in_=kp[:, 256:512])

            def emit_v():
                for sc in range(4):
                    vp = ps_big.tile([128, E], f32, tag="big", name="vp")
                    for ke in range(4):
                        nc.tensor.matmul(
                            vp[:], xeT[ke][:, 128 * sc:128 * sc + 128],
                            wv_sb[:, ke, :],
                            start=(ke == 0), stop=(ke == 3))
                    nc.scalar.copy(
                        out=v_sb[:, sc, :, :],
                        in_=vp[:].rearrange("p (h f) -> p h f", h=H))

            # per (ca): valid query sub-blocks and the triangular mask.
            #   ca0: s0 triangular(kc>=q), s1 empty
            #   ca1: s0 full, s1 triangular(kc>=q)
            #   ca2: s0 triangular(kc<=q), s1 full
            #   ca3: s0 empty, s1 triangular(kc<=q)
            QSL = [(0, 128), (0, 256), (0, 256), (128, 256)]
            TRI = [(0, 1), (1, 1), (0, -1), (1, -1)]  # (s_tri, sign)

            def emit_scores(j):
                # the two heads' scores run row-tiled (K=64 at row groups 0
                # and 64) but must land in separate PSUM banks: two matmul
                # groups writing one bank fault the PE. One padded sp tile
                # spans two banks (hh stride = full bank) so a single exp
                # covers both heads, halving the ACT per-op overhead.
                p_list = []
                for ca in range(4):
                    lo, hi = QSL[ca]
                    ns = (hi - lo) // 128
                    s0 = lo // 128
                    p_t = ptiles.tile([128, 2, 2, 128], bf16, tag="p",
                                      name="p_t")  # [key, s, hh, q]
                    sp = ps_sp.tile([128, 2, SC], f32, tag="sp", name="sp",
                                    padded_shape=[128, 2, 512])
                    for hh in range(2):
                        o = 64 * hh
                        nc.tensor.matmul(
                            sp[:, hh, lo:hi],
                            kT[o:o + 64, j, 128 * ca:128 * ca + 128],
                            qT[o:o + 64, j, lo:hi],
                            start=True, stop=True)
                    nc.scalar.activation(
                        out=p_t[:, s0:s0 + ns, :, :].rearrange(
                            "p s h q -> p h s q"),
                        in_=sp[:, :, lo:hi].rearrange(
                            "p h (s q) -> p h s q", s=ns),
                        func=Act.Exp)
                    s_tri, sign = TRI[ca]
                    nc.gpsimd.affine_select(
                        out=p_t[:, s_tri, :, :], in_=p_t[:, s_tri, :, :],
                        compare_op=Alu.is_ge, fill=0.0, base=0,
                        channel_multiplier=sign,
                        pattern=[[0, 2], [-sign, 128]])
                    p_list.append(p_t)
                return p_list

            pvps = {}
            dns = {}

            def emit_pv_dn(j, p_list):
                # p@v: one bank, even head rows 0:64, odd rows 64:128.
                # ca order starts with ca1 (full-width) so the start=True
                # matmul covers the whole accumulation region.
                pv_ps = ps_pv.tile([128, SC], f32, tag="pv", name="pv_ps")
                for hh in range(2):
                    h = 2 * j + hh
                    rows = slice(64 * hh, 64 * hh + 64)
                    for i, ca in enumerate((1, 0, 2, 3)):
                        lo, hi = QSL[ca]
                        ns = (hi - lo) // 128
                        s0 = lo // 128
                        nc.tensor.matmul(
                            pv_ps[rows, lo:hi],
                            v_sb[:, ca, h, :],
                            p_list[ca][:, s0:s0 + ns, hh, :],
                            start=(i == 0), stop=(i == 3))
                # denominators for both heads, replicated to all 128
                # partitions by an all-ones stationary operand (no
                # partition broadcast needed): [128, (s, hh, q)]
                dn_ps = ps_dn.tile([128, 512], f32, tag="dn", name="dn_ps")
                for i, ca in enumerate((1, 0, 2, 3)):
                    lo, hi = QSL[ca]
                    ns = (hi - lo) // 128
                    s0 = lo // 128
                    rhs = (p_list[ca][:].rearrange("p s h q -> p (s h q)")
                           if ns == 2 else p_list[ca][:, s0:s0 + ns, :, :])
                    nc.tensor.matmul(
                        dn_ps[:, 256 * s0:256 * s0 + 256 * ns],
                        ones_sb[:], rhs,
                        start=(i == 0), stop=(i == 3))
                pvps[j] = pv_ps
                dns[j] = dn_ps

            def emit_norm(j):
                dn_ps = dns.pop(j)
                pv_ps = pvps.pop(j)
                rec = recp.tile([128, 512], f32, tag="rec", name="rec")
                nc.vector.reciprocal_approx_fast(out=rec[:], in_=dn_ps[:])
                bv = rec[:].rearrange("p (s h q) -> p s h q", s=2, h=2)
                for hh in range(2):
                    rows = slice(64 * hh, 64 * hh + 64)
                    nc.vector.tensor_mul(
                        out=x1T[rows, j, :].rearrange("p (s q) -> p s q", s=2),
                        in0=pv_ps[rows, :].rearrange("p (s q) -> p s q", s=2),
                        in1=bv[rows, :, hh, :])

            # ---- attention, software pipelined ----
            emit_proj(0)
            emit_proj(1)
            scores = [emit_scores(0)]
            emit_v()
            emit_proj(2)
            scores.append(emit_scores(1))
            emit_pv_dn(0, scores[0])
            emit_proj(3)
            scores.append(emit_scores(2))
            emit_norm(0)
            emit_pv_dn(1, scores[1])
            scores.append(emit_scores(3))
            emit_norm(1)
            emit_pv_dn(2, scores[2])
            emit_norm(2)
            emit_dummy(2)
            emit_pv_dn(3, scores[3])
            emit_norm(3)
            # keep the PE clock hot through the norm(3) drain so the FFN
            # runs at full speed
            emit_dummy(6)

            # ---- FFN1 (transposed): f^T[u] = relu(W1_u^T x1 + b1_u) ----
            for u in range(4):
                f1_ps = ps_big.tile([128, SC], f32, tag="big",
                                    name=f"f1_ps{u}")
                for j in range(4):
                    nc.tensor.matmul(
                        f1_ps[:], w1_sb[:, j, u, :], x1T[:, j, :],
                        start=(j == 0), stop=(j == 3))
                nc.scalar.activation(
                    out=f_sbT[:, u, :], in_=f1_ps[:], func=Act.Relu,
                    bias=b1_sb[:, u:u + 1], scale=1.0)

            # ---- FFN2 (transposed) + relu + residual + out DMA ----
            for ec in range(4):
                gp = ps_big.tile([128, SC], f32, tag="big", name="gp")
                for u in range(4):
                    nc.tensor.matmul(
                        gp[:], w2_sb[:, u, ec, :], f_sbT[:, u, :],
                        start=(u == 0), stop=(u == 3))
                gsc = gscp.tile([128, SC], f32, tag="gsc", name="gsc")
                nc.scalar.activation(
                    out=gsc[:], in_=gp[:], func=Act.Relu,
                    bias=b2_sb[:, ec:ec + 1], scale=1.0)
                nc.vector.tensor_add(
                    out=out_sbT[:, ec, :], in0=gsc[:],
                    in1=xeT[ec][:, 128:384])
                nc.sync.dma_start(out_d[ec], out_sbT[:, ec, :])

    nc.compile()
    return nc


def _get_nc():
    if "nc" not in _CACHE:
        _CACHE["nc"] = _build_nc()
    return _CACHE["nc"]


def _pack_weights(Wq, Wk, Wv, W1, b1, W2, b2):
    f16 = np.float16
    Wq = np.asarray(Wq, np.float32)
    Wk = np.asarray(Wk, np.float32)
    Wv = np.asarray(Wv, np.float32)
    W1 = np.asarray(W1, np.float32)
    W2 = np.asarray(W2, np.float32)
    # wq[j, p, ke, f] = Wq[128j+f, 128ke+p]
    wq = np.ascontiguousarray(
        Wq.reshape(4, 128, 4, 128).transpose(0, 3, 2, 1).astype(f16))
    wk = np.ascontiguousarray(
        Wk.reshape(4, 128, 4, 128).transpose(0, 3, 2, 1).astype(f16))
    # wv[p, ke, hf] = Wv[hf, 128ke+p]
    wv = np.ascontiguousarray(
        Wv.reshape(512, 4, 128).transpose(2, 1, 0).astype(f16))
    # w1[j, p, u, f1] = (W1/8)[128u+f1, 128j+p]  (1/sqrt(FEAT) folded in)
    w1 = np.ascontiguousarray(
        (W1 / np.sqrt(np.float32(FEAT))).reshape(4, 128, 4, 128)
        .transpose(2, 3, 0, 1).astype(f16))
    # w2[p, u, ec, e] = W2[128ec+e, 128u+p]
    w2 = np.ascontiguousarray(
        W2.reshape(4, 128, 4, 128).transpose(3, 2, 0, 1).astype(f16))
    b1p = np.ascontiguousarray(
        np.asarray(b1, np.float32).reshape(4, 128).T)
    b2p = np.ascontiguousarray(
        np.asarray(b2, np.float32).reshape(4, 128).T)
    return {"wq": wq, "wk": wk, "wv": wv, "w1": w1, "w2": w2,
            "b1": b1p, "b2": b2p}


def _shard_inputs(x, Wq, Wk, Wv, W1, b1, W2, b2):
    x2 = np.ascontiguousarray(np.asarray(x, dtype=np.float32).reshape(S, E))
    ws = _pack_weights(Wq, Wk, Wv, W1, b1, W2, b2)
    in_maps = []
    for c in range(N_CORES):
        r_, half = c >> 1, c & 1
        eidx = np.arange(256 * half - 128, 256 * half + 384)
        valid = (eidx >= 0) & (eidx < S // DIL)
        xe = np.zeros((EXT, E), np.float32)
        xe[valid] = x2[DIL * eidx[valid] + r_]
        # xeT[p, eo, s] = xe[s, 128eo+p]
        xeT = np.ascontiguousarray(
            xe.T.reshape(4, 128, EXT).transpose(1, 0, 2).astype(np.float16))
        in_maps.append({"xeT": xeT, **ws})
    return in_maps


def _gather_outputs(results):
    out = np.zeros((S, E), np.float32)
    for c in range(N_CORES):
        r_, half = c >> 1, c & 1
        # res "out" is [4, 128, SC] = out^T chunks; -> [SC, E]
        oT = np.asarray(results[c]["out"]).reshape(E, SC)
        i = np.arange(256 * half, 256 * half + SC)
        out[DIL * i + r_] = oT.T
    return out.reshape(1, S, E)


def run(inputs, trace=False, tmpdir=None):
    from concourse import bass_utils
    nc = _get_nc()
    in_maps = _shard_inputs(**inputs)
    res = bass_utils.run_bass_kernel_spmd(
        nc, in_maps, list(range(N_CORES)), trace=trace, tmpdir=tmpdir)
    return _gather_outputs(res.results), res


def kernel(x, Wq, Wk, Wv, W1, b1, W2, b2):
    out, _ = run(dict(x=x, Wq=Wq, Wk=Wk, Wv=Wv, W1=W1, b1=b1, W2=W2, b2=b2))
    return out


# revision 29
# speedup vs baseline: 1.1852x; 1.1162x over previous
"""LongFormer dilated-window attention block on 8 trn2 NeuronCores.

Sharding: 8 cores = 4 dilation residues x 2 sequence halves. Query q
attends keys q + 4*j - 512 (j=0..256), i.e. only keys with the same
residue mod DIL=4. De-interleaving by residue turns the dilated window
into a contiguous +-128 sliding window in "residue space". Each core
gets a zero-padded [512, 512] slice of x: its 256 owned rows plus a
128-row halo on each side (all in residue space), so no communication
is needed.

v2 design (vs the fp32 v1 baseline):
- Host pre-casts x and all weights to fp16 and pre-packs them into the
  exact SBUF layouts (x pre-transposed to [e, seq]; weights pre-tiled
  per stationary 128x128 tile). Halves HBM traffic, turns every DMA
  into contiguous >=1KB partition lines, and eliminates all on-chip PE
  transposes and the identity constant.
- All matmuls run fp16 (bf16 for softmax probabilities, which can
  reach exp(44) and overflow fp16). fp16/bf16 weights enable fast
  weight load; fp32 weights do not.
- Scores/p@v are trimmed to the 3 valid 128-key diagonal blocks per
  128-query block (the +-128 window spans 384 keys, not 512).
- Both heads of a pair pack into one PSUM bank: scores as one
  2-row-group matmul group [keys, 2, q]; p@v as even rows 0:64 /
  odd rows 64:128 (col-grp tile_position).
- Softmax denominators via ones-vector matmuls into a [1, 512] bank;
  normalization = DVE reciprocal -> gpsimd partition_broadcast -> DVE
  muls (no PE broadcast/shift matmuls). 1/sqrt(d) is folded into W1
  host-side (FFN1 is linear in x1).
- FFN runs fully transposed ([f, q] layouts) so FFN1's output feeds
  FFN2 directly with no transposes; output is written transposed and
  the host transposes it back. Residual comes from the x^T tile.
- A few warm-up matmuls on a zeroed tile run during the input DMAs to
  lift the PE out of its cold 1.2GHz HAM state before real work.
"""

import sys

if "/opt/trn_rl_repo" not in sys.path:
    sys.path.insert(0, "/opt/trn_rl_repo")

import numpy as np

N_CORES = 8
S, E, H, FEAT = 2048, 512, 8, 64
DIL = 4
SC = 256      # owned queries per core (residue space)
EXT = 512     # ext rows per core (owned + 128 halo each side)

_CACHE = {}


def _build_nc():
    import concourse.bacc as bacc
    import concourse.tile as tile
    import concourse.mybir as mybir
    import concourse.bass as bass

    dt = mybir.dt
    f32 = dt.float32
    f16 = dt.float16
    bf16 = dt.bfloat16
    Alu = mybir.AluOpType
    Act = mybir.ActivationFunctionType

    nc = bacc.Bacc("TRN2", target_bir_lowering=False, debug=False,
                   num_devices=N_CORES)

    # ---- DRAM I/O (host pre-packed, fp16) ----
    xeT_d = nc.dram_tensor("xeT", [128, 4, EXT], f16, kind="ExternalInput").ap()
    wq_d = nc.dram_tensor("wq", [4, 128, 4, 128], f16, kind="ExternalInput").ap()
    wk_d = nc.dram_tensor("wk", [4, 128, 4, 128], f16, kind="ExternalInput").ap()
    wv_d = nc.dram_tensor("wv", [128, 4, E], f16, kind="ExternalInput").ap()
    w1_d = nc.dram_tensor("w1", [4, 128, 4, 128], f16, kind="ExternalInput").ap()
    w2_d = nc.dram_tensor("w2", [128, 4, 4, 128], f16, kind="ExternalInput").ap()
    b1_d = nc.dram_tensor("b1", [128, 4], f32, kind="ExternalInput").ap()
    b2_d = nc.dram_tensor("b2", [128, 4], f32, kind="ExternalInput").ap()
    out_d = nc.dram_tensor("out", [4, 128, SC], f16, kind="ExternalOutput").ap()

    with tile.TileContext(nc) as tc:
        with (
            tc.tile_pool(name="singles", bufs=1) as singles,
            tc.tile_pool(name="ptiles", bufs=8) as ptiles,
            tc.tile_pool(name="recp", bufs=2) as recp,
            tc.tile_pool(name="gscp", bufs=2) as gscp,
            tc.tile_pool(name="ps_big", bufs=2, space="PSUM") as ps_big,
            tc.tile_pool(name="ps_sp", bufs=2, space="PSUM") as ps_sp,
            tc.tile_pool(name="ps_pv", bufs=1, space="PSUM") as ps_pv,
            tc.tile_pool(name="ps_dn", bufs=1, space="PSUM") as ps_dn,
        ):
            # ---- persistent SBUF tiles ----
            # xeT as 4 independent tiles so each DMA chunk unblocks its
            # consumers without waiting for the whole tensor.
            xeT = [singles.tile([128, EXT], f16, name=f"xeT{ke}")
                   for ke in range(4)]
            wq_sb = singles.tile([128, 4, 4, 128], f16)  # [p, j, ke, f]
            wk_sb = singles.tile([128, 4, 4, 128], f16)
            wv_sb = singles.tile([128, 4, E], f16)       # [p, ke, (h f)]
            w1_sb = singles.tile([128, 4, 4, 128], f16)  # [p, j, u, f1]
            w2_sb = singles.tile([128, 4, 4, 128], f16)  # [p, u, ec, e]
            b1_sb = singles.tile([128, 4], f32)
            b2_sb = singles.tile([128, 4], f32)
            qT = singles.tile([128, 4, SC], f16)         # [f_pair, j, q]
            kT = singles.tile([128, 4, EXT], f16)        # [f_pair, j, key]
            v_sb = singles.tile([128, 4, H, FEAT], bf16)  # [key_chunk, sc, h, f]
            x1T = singles.tile([128, 4, SC], f16)        # [f_pair, j, q]
            f_sbT = singles.tile([128, 4, SC], f16)      # [f1_chunk, u, q]
            out_sbT = singles.tile([128, 4, SC], f16)    # [e_chunk, ec, q]
            ones_sb = singles.tile([128, 128], bf16)
            wu_sb = singles.tile([128, 512], f16)

            # ---- input DMAs, in consumption order ----
            nc.sync.dma_start(wq_sb[:, 0], wq_d[0])
            nc.sync.dma_start(xeT[0][:], xeT_d[:, 0, :])
            nc.sync.dma_start(wk_sb[:, 0], wk_d[0])
            for ke in range(1, 4):
                nc.sync.dma_start(xeT[ke][:], xeT_d[:, ke, :])
            nc.sync.dma_start(wq_sb[:, 1:4], wq_d[1:4].rearrange(
                "j p k f -> p j k f"))
            nc.sync.dma_start(wk_sb[:, 1:4], wk_d[1:4].rearrange(
                "j p k f -> p j k f"))
            nc.sync.dma_start(wv_sb[:], wv_d[:])
            nc.sync.dma_start(w1_sb[:], w1_d[:].rearrange(
                "j p k f -> p j k f"))
            nc.sync.dma_start(w2_sb[:], w2_d[:])
            nc.sync.dma_start(b1_sb[:], b1_d[:])
            nc.sync.dma_start(b2_sb[:], b2_d[:])

            # ---- constants ----
            nc.gpsimd.memset(wu_sb[:], 0.0)
            nc.gpsimd.memset(ones_sb[:], 1.0)

            # ---- PE warm-up: matmuls on zeros, results never read.
            # The PE only reaches 2.4GHz after ~3us of *continuous*
            # execution; these bridge the input-DMA wait so real matmuls
            # start (and stay) at full clock.
            def emit_dummy(n):
                for _ in range(n):
                    wu_ps = ps_big.tile([128, 512], f32, tag="big",
                                        name="wu_ps")
                    nc.tensor.matmul(wu_ps[:], wu_sb[:, 0:128], wu_sb[:],
                                     start=True, stop=True)

            emit_dummy(8)

            def emit_proj(j):
                # qT/kT feature chunk of head pair j (heads 2j, 2j+1)
                qp = ps_big.tile([128, SC], f32, tag="big", name="qp")
                for ke in range(4):
                    nc.tensor.matmul(
                        qp[:], wq_sb[:, j, ke, :], xeT[ke][:, 128:384],
                        start=(ke == 0), stop=(ke == 3))
                nc.vector.tensor_copy(out=qT[:, j, :], in_=qp[:])
                kp = ps_big.tile([128, EXT], f32, tag="big", name="kp")
                for ke in range(4):
                    nc.tensor.matmul(
                        kp[:], wk_sb[:, j, ke, :], xeT[ke][:],
                        start=(ke == 0), stop=(ke == 3))
                # halves, so scores ca0/ca1 gate only on the first 256 keys
                nc.vector.tensor_copy(out=kT[:, j, 0:256], in_=kp[:, 0:256])
                nc.vector.tensor_copy(out=kT[:, j, 256:512],
                                      in_=kp[:, 256:512])

            def emit_v():
                # copies split across ACT and DVE so neither engine's
                # in-order queue stalls the exp/cast streams for long
                for sc in range(4):
                    vp = ps_big.tile([128, E], f32, tag="big", name="vp")
                    for ke in range(4):
                        nc.tensor.matmul(
                            vp[:], xeT[ke][:, 128 * sc:128 * sc + 128],
                            wv_sb[:, ke, :],
                            start=(ke == 0), stop=(ke == 3))
                    eng = nc.scalar.copy if sc % 2 == 0 else (
                        lambda out, in_: nc.vector.tensor_copy(out=out,
                                                               in_=in_))
                    eng(out=v_sb[:, sc, :, :],
                        in_=vp[:].rearrange("p (h f) -> p h f", h=H))

            # per (ca): valid query sub-blocks and the triangular mask.
            #   ca0: s0 triangular(kc>=q), s1 empty
            #   ca1: s0 full, s1 triangular(kc>=q)
            #   ca2: s0 triangular(kc<=q), s1 full
            #   ca3: s0 empty, s1 triangular(kc<=q)
            QSL = [(0, 128), (0, 256), (0, 256), (128, 256)]
            TRI = [(0, 1), (1, 1), (0, -1), (1, -1)]  # (s_tri, sign)

            def emit_scores(j):
                # the two heads' scores run row-tiled (K=64 at row groups 0
                # and 64) but must land in separate PSUM banks: two matmul
                # groups writing one bank fault the PE. One padded sp tile
                # spans two banks (hh stride = full bank) so a single exp
                # covers both heads, halving the ACT per-op overhead.
                p_list = []
                for ca in range(4):
                    lo, hi = QSL[ca]
                    ns = (hi - lo) // 128
                    s0 = lo // 128
                    p_t = ptiles.tile([128, 2, 2, 128], bf16, tag="p",
                                      name="p_t")  # [key, s, hh, q]
                    sp = ps_sp.tile([128, 2, SC], f32, tag="sp", name="sp",
                                    padded_shape=[128, 2, 512])
                    for hh in range(2):
                        o = 64 * hh
                        nc.tensor.matmul(
                            sp[:, hh, lo:hi],
                            kT[o:o + 64, j, 128 * ca:128 * ca + 128],
                            qT[o:o + 64, j, lo:hi],
                            start=True, stop=True)
                    nc.scalar.activation(
                        out=p_t[:, s0:s0 + ns, :, :].rearrange(
                            "p s h q -> p h s q"),
                        in_=sp[:, :, lo:hi].rearrange(
                            "p h (s q) -> p h s q", s=ns),
                        func=Act.Exp)
                    s_tri, sign = TRI[ca]
                    nc.gpsimd.affine_select(
                        out=p_t[:, s_tri, :, :], in_=p_t[:, s_tri, :, :],
                        compare_op=Alu.is_ge, fill=0.0, base=0,
                        channel_multiplier=sign,
                        pattern=[[0, 2], [-sign, 128]])
                    p_list.append(p_t)
                return p_list

            pvps = {}
            dns = {}

            def emit_pv_dn(j, p_list):
                # p@v: one bank, even head rows 0:64, odd rows 64:128.
                # ca order starts with ca1 (full-width) so the start=True
                # matmul covers the whole accumulation region.
                pv_ps = ps_pv.tile([128, SC], f32, tag="pv", name="pv_ps")
                for hh in range(2):
                    h = 2 * j + hh
                    rows = slice(64 * hh, 64 * hh + 64)
                    for i, ca in enumerate((1, 0, 2, 3)):
                        lo, hi = QSL[ca]
                        ns = (hi - lo) // 128
                        s0 = lo // 128
                        nc.tensor.matmul(
                            pv_ps[rows, lo:hi],
                            v_sb[:, ca, h, :],
                            p_list[ca][:, s0:s0 + ns, hh, :],
                            start=(i == 0), stop=(i == 3))
                # denominators for both heads, replicated to all 128
                # partitions by an all-ones stationary operand (no
                # partition broadcast needed): [128, (s, hh, q)]
                dn_ps = ps_dn.tile([128, 512], f32, tag="dn", name="dn_ps")
                for i, ca in enumerate((1, 0, 2, 3)):
                    lo, hi = QSL[ca]
                    ns = (hi - lo) // 128
                    s0 = lo // 128
                    rhs = (p_list[ca][:].rearrange("p s h q -> p (s h q)")
                           if ns == 2 else p_list[ca][:, s0:s0 + ns, :, :])
                    nc.tensor.matmul(
                        dn_ps[:, 256 * s0:256 * s0 + 256 * ns],
                        ones_sb[:], rhs,
                        start=(i == 0), stop=(i == 3))
                pvps[j] = pv_ps
                dns[j] = dn_ps

            def emit_norm(j):
                dn_ps = dns.pop(j)
                pv_ps = pvps.pop(j)
                rec = recp.tile([128, 512], f32, tag="rec", name="rec")
                nc.vector.reciprocal_approx_fast(out=rec[:], in_=dn_ps[:])
                bv = rec[:].rearrange("p (s h q) -> p s h q", s=2, h=2)
                for hh in range(2):
                    rows = slice(64 * hh, 64 * hh + 64)
                    nc.vector.tensor_mul(
                        out=x1T[rows, j, :].rearrange("p (s q) -> p s q", s=2),
                        in0=pv_ps[rows, :].rearrange("p (s q) -> p s q", s=2),
                        in1=bv[rows, :, hh, :])

            # ---- attention, software pipelined ----
            emit_proj(0)
            emit_proj(1)
            scores = [emit_scores(0)]
            emit_v()
            emit_proj(2)
            scores.append(emit_scores(1))
            emit_pv_dn(0, scores[0])
            emit_proj(3)
            scores.append(emit_scores(2))
            emit_norm(0)
            emit_pv_dn(1, scores[1])
            scores.append(emit_scores(3))
            emit_norm(1)
            emit_pv_dn(2, scores[2])
            emit_norm(2)
            emit_dummy(2)
            emit_pv_dn(3, scores[3])
            emit_norm(3)
            # keep the PE clock hot through the norm(3) drain so the FFN
            # runs at full speed
            emit_dummy(6)

            # ---- FFN1 (transposed): f^T[u] = relu(W1_u^T x1 + b1_u) ----
            for u in range(4):
                f1_ps = ps_big.tile([128, SC], f32, tag="big",
                                    name=f"f1_ps{u}")
                for j in range(4):
                    nc.tensor.matmul(
                        f1_ps[:], w1_sb[:, j, u, :], x1T[:, j, :],
                        start=(j == 0), stop=(j == 3))
                nc.scalar.activation(
                    out=f_sbT[:, u, :], in_=f1_ps[:], func=Act.Relu,
                    bias=b1_sb[:, u:u + 1], scale=1.0)

            # ---- FFN2 (transposed) + relu + residual + out DMA ----
            for ec in range(4):
                gp = ps_big.tile([128, SC], f32, tag="big", name="gp")
                for u in range(4):
                    nc.tensor.matmul(
                        gp[:], w2_sb[:, u, ec, :], f_sbT[:, u, :],
                        start=(u == 0), stop=(u == 3))
                gsc = gscp.tile([128, SC], f32, tag="gsc", name="gsc")
                nc.scalar.activation(
                    out=gsc[:], in_=gp[:], func=Act.Relu,
                    bias=b2_sb[:, ec:ec + 1], scale=1.0)
                nc.vector.tensor_add(
                    out=out_sbT[:, ec, :], in0=gsc[:],
                    in1=xeT[ec][:, 128:384])
                nc.sync.dma_start(out_d[ec], out_sbT[:, ec, :])

    nc.compile()
    return nc


def _get_nc():
    if "nc" not in _CACHE:
        _CACHE["nc"] = _build_nc()
    return _CACHE["nc"]


def _pack_weights(Wq, Wk, Wv, W1, b1, W2, b2):
    f16 = np.float16
    Wq = np.asarray(Wq, np.float32)
    Wk = np.asarray(Wk, np.float32)
    Wv = np.asarray(Wv, np.float32)
    W1 = np.asarray(W1, np.float32)
    W2 = np.asarray(W2, np.float32)
    # wq[j, p, ke, f] = Wq[128j+f, 128ke+p]
    wq = np.ascontiguousarray(
        Wq.reshape(4, 128, 4, 128).transpose(0, 3, 2, 1).astype(f16))
    wk = np.ascontiguousarray(
        Wk.reshape(4, 128, 4, 128).transpose(0, 3, 2, 1).astype(f16))
    # wv[p, ke, hf] = Wv[hf, 128ke+p]
    wv = np.ascontiguousarray(
        Wv.reshape(512, 4, 128).transpose(2, 1, 0).astype(f16))
    # w1[j, p, u, f1] = (W1/8)[128u+f1, 128j+p]  (1/sqrt(FEAT) folded in)
    w1 = np.ascontiguousarray(
        (W1 / np.sqrt(np.float32(FEAT))).reshape(4, 128, 4, 128)
        .transpose(2, 3, 0, 1).astype(f16))
    # w2[p, u, ec, e] = W2[128ec+e, 128u+p]
    w2 = np.ascontiguousarray(
        W2.reshape(4, 128, 4, 128).transpose(3, 2, 0, 1).astype(f16))
    b1p = np.ascontiguousarray(
        np.asarray(b1, np.float32).reshape(4, 128).T)
    b2p = np.ascontiguousarray(
        np.asarray(b2, np.float32).reshape(4, 128).T)
    return {"wq": wq, "wk": wk, "wv": wv, "w1": w1, "w2": w2,
            "b1": b1p, "b2": b2p}


def _shard_inputs(x, Wq, Wk, Wv, W1, b1, W2, b2):
    x2 = np.ascontiguousarray(np.asarray(x, dtype=np.float32).reshape(S, E))
    ws = _pack_weights(Wq, Wk, Wv, W1, b1, W2, b2)
    in_maps = []
    for c in range(N_CORES):
        r_, half = c >> 1, c & 1
        eidx = np.arange(256 * half - 128, 256 * half + 384)
        valid = (eidx >= 0) & (eidx < S // DIL)
        xe = np.zeros((EXT, E), np.float32)
        xe[valid] = x2[DIL * eidx[valid] + r_]
        # xeT[p, eo, s] = xe[s, 128eo+p]
        xeT = np.ascontiguousarray(
            xe.T.reshape(4, 128, EXT).transpose(1, 0, 2).astype(np.float16))
        in_maps.append({"xeT": xeT, **ws})
    return in_maps


def _gather_outputs(results):
    out = np.zeros((S, E), np.float32)
    for c in range(N_CORES):
        r_, half = c >> 1, c & 1
        # res "out" is [4, 128, SC] = out^T chunks; -> [SC, E]
        oT = np.asarray(results[c]["out"]).reshape(E, SC)
        i = np.arange(256 * half, 256 * half + SC)
        out[DIL * i + r_] = oT.T
    return out.reshape(1, S, E)


def run(inputs, trace=False, tmpdir=None):
    from concourse import bass_utils
    nc = _get_nc()
    in_maps = _shard_inputs(**inputs)
    res = bass_utils.run_bass_kernel_spmd(
        nc, in_maps, list(range(N_CORES)), trace=trace, tmpdir=tmpdir)
    return _gather_outputs(res.results), res


def kernel(x, Wq, Wk, Wv, W1, b1, W2, b2):
    out, _ = run(dict(x=x, Wq=Wq, Wk=Wk, Wv=Wv, W1=W1, b1=b1, W2=W2, b2=b2))
    return out
